# revision 61
# baseline (speedup 1.0000x reference)
"""Distributed Trainium2 kernel for a dense transformer block.

Reference computation (per batch):
  x = x + o_proj(attn(rope(qkv(rmsnorm(x))), causal)) ; x = x + w2(silu(wg(rmsnorm(x))) * w1(rmsnorm(x)))

Sharding: DP=2 on batch x TP=4 on heads / MLP rows (Megatron).
Cores 0-3 handle batch 0, cores 4-7 batch 1. Within a group, rank r owns
heads 4r..4r+3 and MLP rows 1024r..1024(r+1).

Collective strategy (v2): instead of two AllReduces, use
  - AllGather of the normalized per-head attention outputs (feature-major,
    256KB per rank per 512-token chunk); every rank then computes the FULL
    o-proj locally.  Halves the wire bytes and runs in one mesh phase.
  - ReduceScatter of the MLP output + h/4 residual; each rank ends up with
    a 128-token slice per chunk and Python reassembles the full output from
    all 8 cores.
Both are chunked into 4 pieces and software-pipelined against compute.
"""

import sys

sys.path.insert(0, "/opt/trn_rl_repo")

import numpy as np
import ml_dtypes

import concourse.bass as bass
import concourse.bacc as bacc
import concourse.mybir as mybir
import concourse.tile as tile
from concourse.bass_utils import run_bass_kernel_spmd

BF = ml_dtypes.bfloat16
F32 = mybir.dt.float32
BF16 = mybir.dt.bfloat16

D = 1024
NH = 16
DH = 64
MULT = 4
EPS = 1e-5
ROPE_BASE = 10000.0
B = 2
TP = 4  # tensor-parallel ranks per group
HPC = NH // TP  # heads per core = 4
QKF = 2 * HPC * DH  # q+k shard features = 512
VF = HPC * DH  # v shard features = 256
MID = MULT * D // TP  # mlp rows per core = 1024
AF = mybir.ActivationFunctionType
ALU = mybir.AluOpType


def build_nc(T, use_silu=False):
    """Build the SPMD graph for one core (token count T per batch)."""
    DC = D // 128  # d chunks = 8
    TT = T // 128  # token tiles
    QT = min(512, T)  # q-tile width == collective chunk width
    NQ = T // QT
    CPQ = QT // 128  # 128-token tiles per chunk
    MIDC = MID // 128  # mlp row chunks = 8
    NT = D // 512

    nc = bacc.Bacc("TRN2", target_bir_lowering=False, debug=False, num_devices=8)

    x_e = nc.dram_tensor("x", [T, D], F32, kind="ExternalInput")
    qkw_e = nc.dram_tensor("qkw_t", [D, QKF], BF16, kind="ExternalInput")
    vw_e = nc.dram_tensor("vw_m", [D, VF], BF16, kind="ExternalInput")
    ow_e = nc.dram_tensor("ow_m", [D, D], BF16, kind="ExternalInput")
    w1w_e = nc.dram_tensor("w1w_t", [D, MID], BF16, kind="ExternalInput")
    wgw_e = nc.dram_tensor("wgw_t", [D, MID], BF16, kind="ExternalInput")
    w2w_e = nc.dram_tensor("w2w_m", [MID, D], BF16, kind="ExternalInput")
    cos_e = nc.dram_tensor("cosr", [128, T], BF16, kind="ExternalInput")
    sin_e = nc.dram_tensor("sinr", [128, T], BF16, kind="ExternalInput")
    cm_e = nc.dram_tensor("cmask", [128, 128], BF16, kind="ExternalInput")
    id_e = nc.dram_tensor("ident", [128, 128], BF16, kind="ExternalInput")
    out_e = nc.dram_tensor("out", [NQ * 128, D], F32, kind="ExternalOutput")

    groups = [[0, 1, 2, 3], [4, 5, 6, 7]]

    with tile.TileContext(nc) as tc:
        with (
            tc.tile_pool(name="const", bufs=1) as cpool,
            tc.tile_pool(name="actfm", bufs=1) as fmpool,
            tc.tile_pool(name="qko", bufs=1) as qkpool,
            tc.tile_pool(name="vaug", bufs=1) as vpool,
            tc.tile_pool(name="xin", bufs=4) as xpool,
            tc.tile_pool(name="xnb", bufs=3) as xnpool,
            tc.tile_pool(name="work", bufs=4) as wpool,
            tc.tile_pool(name="rope", bufs=2) as rpool,
            tc.tile_pool(name="stats", bufs=8) as spool,
            tc.tile_pool(name="hres", bufs=8) as hpool,
            tc.tile_pool(name="agos", bufs=2) as agpool,
            tc.tile_pool(name="psA", bufs=4, space="PSUM") as psA,
            tc.tile_pool(name="psO", bufs=2, space="PSUM") as psO,
            tc.tile_pool(name="psS", bufs=2, space="PSUM") as psS,
            tc.tile_pool(name="dram", bufs=1, space="DRAM") as dpool,
        ):
            # ---- resident weights / tables ----
            def load_tiles(src, width, n, dt=BF16):
                ts = []
                for i in range(n):
                    t = cpool.tile(
                        [128, width], dt, tag=f"{src.name}_{i}", name=f"{src.name}_{i}"
                    )
                    nc.sync.dma_start(t[:], src[i * 128 : (i + 1) * 128, :])
                    ts.append(t)
                return ts

            # ident + epsc + chunk-0 x tiles first: the norm1 chain for
            # chunk 0 doesn't wait behind the weight stack's DMAs.
            ident = load_tiles(id_e, 128, 1)[0]
            epsc = cpool.tile([128, 1], F32, tag="epsc", name="epsc")
            nc.vector.memset(epsc[:], EPS)
            x0_tiles = []
            for tau in range(CPQ):
                xt = xpool.tile([128, D], F32, tag="xt", name="xt")
                nc.sync.dma_start(xt[:], x_e[tau * 128 : (tau + 1) * 128, :])
                x0_tiles.append(xt)

            qkw = load_tiles(qkw_e, QKF, DC)
            vw = load_tiles(vw_e, VF, DC)
            ow = load_tiles(ow_e, D, DC)
            w1r = w1w_e.rearrange("(c p) m -> p c m", p=128)
            wgr = wgw_e.rearrange("(c p) m -> p c m", p=128)
            cos_t, sin_t = [], []
            for t4 in range(NQ):
                for src, dst in ((cos_e, cos_t), (sin_e, sin_t)):
                    t = cpool.tile(
                        [128, QT], BF16, tag=f"{src.name}_{t4}",
                        name=f"{src.name}c{t4}",
                    )
                    nc.sync.dma_start(t[:], src[:, t4 * QT : (t4 + 1) * QT])
                    dst.append(t)
            tri = load_tiles(cm_e, 128, 1)[0]
            ones64 = cpool.tile([1, 64], BF16, tag="ones64", name="ones64")
            nc.vector.memset(ones64[:], 1.0)

            ag_in = [
                dpool.tile([VF, QT], BF16, name=f"ag_in{k}") for k in range(NQ)
            ]
            ag_out = [
                dpool.tile([TP, VF, QT], BF16, name=f"ag_out{k}")
                for k in range(NQ)
            ]
            rs_in = [
                dpool.tile([QT, D], BF16, name=f"rs_in{k}") for k in range(NQ)
            ]
            rs_out = [
                dpool.tile([QT // TP, D], BF16, name=f"rs_out{k}")
                for k in range(NQ)
            ]

            # ---- persistent activation tiles ----
            # chunk-local normalized activations, feature-major:
            # fm[:, dc, tau*128:...] holds (x-hat chunk)^T for d-block dc
            xnf_c = [
                fmpool.tile(
                    [128, DC, QT], BF16, tag="fm", name=f"xnf{t4}", bufs=2
                )
                for t4 in range(NQ)
            ]
            q_sb = [
                qkpool.tile([128, T], BF16, tag=f"qk{i}", name=f"q{i}")
                for i in range(2)
            ]
            k_sb = [
                qkpool.tile([128, T], BF16, tag=f"qk{i + 2}", name=f"k{i}")
                for i in range(2)
            ]
            O_sb = [
                qkpool.tile([128, T], BF16, tag=f"qk{i + 4}", name=f"O{i}")
                for i in range(2)
            ]
            On_sb = [
                qkpool.tile([128, T], BF16, tag=f"qk{i + 6}", name=f"On{i}")
                for i in range(2)
            ]
            v_aug = [
                vpool.tile([128, HPC, DH + 1], BF16, tag=f"va{ti}", name=f"va{ti}")
                for ti in range(TT)
            ]

            # ---- helpers ----
            def norm_into_fm(xt, fm_c, tau):
                """rmsnorm the token tile xt, write bf16 feature-major via
                a single DMA-transpose into fm_c[:, :, tau*128:(tau+1)*128]."""
                ss = spool.tile([128, 1], F32, tag="ss", name="ss")
                sq = xnpool.tile([128, D], BF16, tag="sq", name="sq", bufs=1)
                nc.vector.scalar_tensor_tensor(
                    sq[:], xt[:], 1.0, xt[:], ALU.mult, ALU.mult, accum_out=ss[:]
                )
                sr = spool.tile([128, 1], F32, tag="sr", name="sr")
                nc.scalar.activation(
                    out=sr[:], in_=ss[:], func=AF.Sqrt, bias=epsc[:], scale=1.0 / D
                )
                s1 = spool.tile([128, 1], F32, tag="s1", name="s1")
                nc.vector.reciprocal(s1[:], sr[:])
                xn = xnpool.tile([128, D], BF16, tag="xn", name="xn")
                nc.vector.tensor_scalar_mul(xn[:], xt[:], s1[:])
                for di in range(DC):
                    tp = psS.tile([128, 128], BF16, tag="tp", name="tp", bufs=2)
                    nc.tensor.transpose(
                        tp[:], xn[:, di * 128 : (di + 1) * 128], ident[:]
                    )
                    nc.any.tensor_copy(
                        fm_c[:, di, tau * 128 : (tau + 1) * 128], tp[:]
                    )

            # ---- stage A/B interleaved: norm1 + qkv + rope per chunk ----
            def norm1_t4(t4):
                for tau in range(CPQ):
                    ti = t4 * CPQ + tau
                    if t4 == 0:
                        xt = x0_tiles[tau]
                    else:
                        xt = xpool.tile([128, D], F32, tag="xt", name="xt")
                        nc.sync.dma_start(
                            xt[:], x_e[ti * 128 : (ti + 1) * 128, :]
                        )
                    norm_into_fm(xt, xnf_c[t4], tau)

            def qk_t4(t4):
                tsl = slice(t4 * QT, (t4 + 1) * QT)
                for m in range(4):  # q01 q23 k01 k23
                    dst = q_sb[m] if m < 2 else k_sb[m - 2]
                    ps = psA.tile(
                        [128, 512 if T >= 512 else T], F32, tag="ps", name="ps"
                    )
                    for dc in range(DC):
                        nc.tensor.matmul(
                            ps[:, :QT],
                            qkw[dc][:, m * 128 : (m + 1) * 128],
                            xnf_c[t4][:, dc, :],
                            start=(dc == 0),
                            stop=(dc == DC - 1),
                        )
                    qb = rpool.tile([128, QT], BF16, tag="qb", name="qb")
                    nc.scalar.copy(qb[:], ps[:, :QT])
                    rot = rpool.tile([128, QT], BF16, tag="rot", name="rot")
                    for hb in (0, 64):
                        nc.vector.tensor_scalar_mul(
                            rot[hb : hb + 32, :], qb[hb + 32 : hb + 64, :], -1.0
                        )
                        nc.vector.tensor_copy(
                            rot[hb + 32 : hb + 64, :], qb[hb : hb + 32, :]
                        )
                    t1 = rpool.tile([128, QT], BF16, tag="t1", name="t1")
                    nc.vector.tensor_mul(t1[:], qb[:], cos_t[t4][:])
                    t2 = rpool.tile([128, QT], BF16, tag="t2", name="t2")
                    nc.vector.tensor_mul(t2[:], rot[:], sin_t[t4][:])
                    nc.vector.tensor_add(dst[:, tsl], t1[:], t2[:])

            def v_chunk(t4):
                for tau in range(CPQ):
                    ti = t4 * CPQ + tau
                    ps = psS.tile([128, VF], F32, tag="tp", name="psv")
                    for dc in range(DC):
                        nc.tensor.matmul(
                            ps[:],
                            xnf_c[t4][:, dc, tau * 128 : (tau + 1) * 128],
                            vw[dc][:],
                            start=(dc == 0),
                            stop=(dc == DC - 1),
                        )
                    va = v_aug[ti]
                    nc.vector.tensor_copy(
                        va[:, :, 0:DH], ps.rearrange("p (h d) -> p h d", h=HPC)
                    )
                    nc.vector.memset(va[:, :, DH : DH + 1], 1.0)

            # ---- stage C: attention + AllGather of normalized head outputs ----
            def attn_qtile(qt):
                tsl = slice(qt * QT, (qt + 1) * QT)
                ncks = CPQ * (qt + 1)
                dnb = spool.tile(
                    [1, HPC * QT], BF16, tag="dnb", name="dnb", bufs=1
                )
                for hp in range(2):
                    opsP = [
                        psO.tile([DH + 1, QT], F32, tag="pso", name=f"ops{i}")
                        for i in range(2)
                    ]

                    def emit_scores(ck):
                        j = ck - CPQ * qt  # >=0 inside the diagonal block
                        lo = max(j, 0) * 128
                        pts = []
                        for i in range(2):
                            hb = i * 64
                            sp = psA.tile(
                                [128, 512 if T >= 512 else T], F32, tag="ps", name="sp"
                            )
                            nc.tensor.matmul(
                                sp[:, :QT],
                                k_sb[hp][hb : hb + DH, ck * 128 : (ck + 1) * 128],
                                q_sb[hp][hb : hb + DH, tsl],
                                start=True,
                                stop=True,
                            )
                            pt = wpool.tile(
                                [128, QT], BF16, tag="pt", name="pt", bufs=6
                            )
                            if j > 0:
                                nc.vector.memset(pt[:, :lo], 0.0)
                            nc.scalar.activation(
                                out=pt[:, lo:],
                                in_=sp[:, lo:QT],
                                func=AF.Exp,
                                scale=0.125,
                            )
                            if j >= 0:
                                nc.vector.tensor_mul(
                                    pt[:, lo : lo + 128], pt[:, lo : lo + 128], tri[:]
                                )
                            pts.append(pt)
                        return pts

                    def emit_av(ck, pts):
                        for i in range(2):
                            nc.tensor.matmul(
                                opsP[i][:],
                                v_aug[ck][:, 2 * hp + i, :],
                                pts[i][:],
                                start=(ck == 0),
                                stop=(ck == ncks - 1),
                            )

                    # scores emitted one chunk ahead of AV so the PE never
                    # waits on the Exp chain
                    prev = emit_scores(0)
                    for ck in range(1, ncks):
                        cur = emit_scores(ck)
                        emit_av(ck - 1, prev)
                        prev = cur
                    emit_av(ncks - 1, prev)
                    for i in range(2):
                        h = 2 * hp + i
                        ops = opsP[i]
                        nc.vector.tensor_copy(
                            dnb[:, h * QT : (h + 1) * QT], ops[DH : DH + 1, :]
                        )
                        nc.scalar.copy(
                            O_sb[hp][i * 64 : i * 64 + DH, tsl], ops[0:DH, :]
                        )
                return dnb

            def normalize_qt(qt, dnb):
                tsl = slice(qt * QT, (qt + 1) * QT)
                for ot in range(2):
                    bb = psA.tile(
                        [128, 512 if T >= 512 else T], F32, tag="ps", name="bb"
                    )
                    for i in range(2):
                        h = 2 * ot + i
                        nc.tensor.matmul(
                            bb[i * 64 : (i + 1) * 64, :QT],
                            ones64[:],
                            dnb[:, h * QT : (h + 1) * QT],
                            start=True,
                            stop=True,
                        )
                    rb = wpool.tile([128, QT], BF16, tag="rb", name="rb", bufs=2)
                    with nc.allow_low_precision(
                        reason="softmax denom ~O(1); bf16 recip matches prior"
                    ):
                        nc.vector.reciprocal(rb[:], bb[:, :QT])
                    nc.vector.tensor_mul(
                        On_sb[ot][:, tsl], O_sb[ot][:, tsl], rb[:]
                    )

            def ag_fire(qt):
                tsl = slice(qt * QT, (qt + 1) * QT)
                for ot in range(2):
                    nc.sync.dma_start(
                        ag_in[qt][ot * 128 : (ot + 1) * 128, :], On_sb[ot][:, tsl]
                    )
                nc.gpsimd.collective_compute(
                    "AllGather",
                    ALU.bypass,
                    ins=[ag_in[qt][:].opt()],
                    outs=[ag_out[qt][:].opt()],
                    replica_groups=groups,
                )

            # ---- stage D: full o-proj + residual + norm2 (per chunk) ----
            hnf_c = [
                fmpool.tile(
                    [128, DC, QT], BF16, tag="fm", name=f"hnf{k}", bufs=2
                )
                for k in range(NQ)
            ]
            hres = [
                hpool.tile([128, D], BF16, tag="hr", name=f"hr{ti}", bufs=8)
                for ti in range(TT)
            ]

            def ago_load(k):
                # gathered normalized head outputs, feature-major [1024, QT]
                ago = agpool.tile(
                    [128, TP, 2, QT], BF16, tag="ago", name=f"ago{k}", bufs=2
                )
                agr = ag_out[k][:].rearrange("r (c p) t -> p r c t", p=128)
                nc.sync.dma_start(ago[:], agr)
                return ago

            def oproj_chunk(k, ago):
                for tau in range(CPQ):
                    ti = k * CPQ + tau
                    xo = xpool.tile([128, D], F32, tag="xo", name="xo")
                    nc.sync.dma_start(xo[:], x_e[ti * 128 : (ti + 1) * 128, :])
                    ob = hres[ti]
                    csl = slice(tau * 128, (tau + 1) * 128)
                    for nt in range(NT):
                        ps = psA.tile([128, 512], F32, tag="ps", name="ps")
                        for c in range(DC):
                            nc.tensor.matmul(
                                ps[:, :512],
                                ago[:, c // 2, c % 2, csl],
                                ow[c][:, nt * 512 : (nt + 1) * 512],
                                start=(c == 0),
                                stop=(c == DC - 1),
                            )
                        nc.vector.scalar_tensor_tensor(
                            ob[:, nt * 512 : (nt + 1) * 512],
                            xo[:, nt * 512 : (nt + 1) * 512],
                            1.0,
                            ps[:, :512],
                            ALU.mult,
                            ALU.add,
                        )
                    norm_into_fm(ob, hnf_c[k], tau)

            # ---- stage E: MLP (chunk-pipelined) + ReduceScatter ----
            # w2 is loaded as 16 half-tiles [128, 512], reusing the SBUF of
            # the (now dead) qkv weights and causal-mask tiles.
            host_tags = (
                [f"qkw_t_{i}" for i in range(DC)]
                + [f"cosr_{i}" for i in range(NQ)]
                + [f"sinr_{i}" for i in range(NQ)]
            )
            w2h = []
            for i in range(2 * MIDC):
                t = cpool.tile(
                    [128, 512], BF16, tag=host_tags[i], name=f"w2h{i}"
                )
                nc.sync.dma_start(
                    t[:],
                    w2w_e[
                        (i // 2) * 128 : (i // 2 + 1) * 128,
                        (i % 2) * 512 : (i % 2 + 1) * 512,
                    ],
                )
                w2h.append(t)
            a_fm = [
                qkpool.tile([128, T], BF16, tag=f"qk{d}", name=f"a{d}")
                for d in range(MIDC)
            ]

            def mlp_t4(t4):
                tsl = slice(t4 * QT, (t4 + 1) * QT)
                for mc in range(MIDC):
                    msl = slice(mc * 128, (mc + 1) * 128)
                    wg_mc = wpool.tile(
                        [128, DC, 128], BF16, tag="wgs", name="wg_mc", bufs=2
                    )
                    nc.sync.dma_start(wg_mc[:], wgr[:, :, msl])
                    w1_mc = wpool.tile(
                        [128, DC, 128], BF16, tag="w1s", name="w1_mc", bufs=2
                    )
                    nc.sync.dma_start(w1_mc[:], w1r[:, :, msl])
                    psg = psA.tile(
                        [128, 512 if T >= 512 else T], F32, tag="ps", name="psg"
                    )
                    for dc in range(DC):
                        nc.tensor.matmul(
                            psg[:, :QT],
                            wg_mc[:, dc, :],
                            hnf_c[t4][:, dc, :],
                            start=(dc == 0),
                            stop=(dc == DC - 1),
                        )
                    g_sb = wpool.tile([128, QT], BF16, tag="g", name="g", bufs=2)
                    if use_silu:
                        nc.scalar.activation(
                            out=g_sb[:], in_=psg[:, :QT], func=AF.Silu
                        )
                    else:
                        sg = wpool.tile([128, QT], F32, tag="sg", name="sg", bufs=2)
                        nc.scalar.activation(
                            out=sg[:], in_=psg[:, :QT], func=AF.Sigmoid
                        )
                        nc.vector.tensor_mul(g_sb[:], sg[:], psg[:, :QT])
                    psu = psA.tile(
                        [128, 512 if T >= 512 else T], F32, tag="ps", name="psu"
                    )
                    for dc in range(DC):
                        nc.tensor.matmul(
                            psu[:, :QT],
                            w1_mc[:, dc, :],
                            hnf_c[t4][:, dc, :],
                            start=(dc == 0),
                            stop=(dc == DC - 1),
                        )
                    nc.vector.tensor_mul(a_fm[mc][:, tsl], g_sb[:], psu[:, :QT])

            def w2_rs(t4):
                for tau in range(CPQ):
                    ti = t4 * CPQ + tau
                    ob = wpool.tile([128, D], BF16, tag="ob", name="ob", bufs=2)
                    for nt in range(NT):
                        ps = psA.tile([128, 512], F32, tag="ps", name="ps")
                        for mc in range(MIDC):
                            nc.tensor.matmul(
                                ps[:, :512],
                                a_fm[mc][:, ti * 128 : (ti + 1) * 128],
                                w2h[2 * mc + nt][:],
                                start=(mc == 0),
                                stop=(mc == MIDC - 1),
                            )
                        nc.vector.scalar_tensor_tensor(
                            ob[:, nt * 512 : (nt + 1) * 512],
                            hres[ti][:, nt * 512 : (nt + 1) * 512],
                            1.0 / TP,
                            ps[:, :512],
                            ALU.mult,
                            ALU.add,
                        )
                    nc.sync.dma_start(
                        rs_in[t4][tau * 128 : (tau + 1) * 128, :],
                        ob[:],
                    )
                    if tau % 2 == 1:
                        # two half-size RS ops per chunk: each fires as soon
                        # as its data is staged, so the serial CC chain never
                        # waits long for local data
                        h = tau // 2
                        nc.gpsimd.collective_compute(
                            "ReduceScatter",
                            ALU.add,
                            ins=[rs_in[t4][h * 256 : (h + 1) * 256, :].opt()],
                            outs=[rs_out[t4][h * 64 : (h + 1) * 64, :].opt()],
                            replica_groups=groups,
                        )

            def final_chunk(k):
                nc.gpsimd.dma_start(
                    out_e[k * 128 : (k + 1) * 128, :],
                    rs_out[k][:],
                )

            # ---- schedule ----
            for t4 in range(NQ):
                norm1_t4(t4)
                qk_t4(t4)
                v_chunk(t4)
            for qt in range(NQ):
                dnb = attn_qtile(qt)
                normalize_qt(qt, dnb)
                ag_fire(qt)
            # stage D runs one chunk ahead of stage E on the PE, so mlp(k)
            # never waits on the norm2 chain of its own chunk
            agos = {0: ago_load(0), 1: ago_load(1)}
            oproj_chunk(0, agos[0])
            for k in range(NQ):
                if k + 1 < NQ:
                    oproj_chunk(k + 1, agos[k + 1])
                mlp_t4(k)
                w2_rs(k)
                if k + 2 < NQ:
                    agos[k + 2] = ago_load(k + 2)
                if k >= 1:
                    final_chunk(k - 1)
            final_chunk(NQ - 1)

    nc.compile()
    return nc


def make_in_maps(x, n1_w, n2_w, qkv_w, o_w, w1_w, wg_w, w2_w, T):
    QT = min(512, T)
    CPQ = QT // 128
    half = DH // 2
    freqs = np.arange(half, dtype=np.float64) / half
    theta = 1.0 / ROPE_BASE**freqs
    ang = np.arange(T, dtype=np.float64)[:, None] * theta[None, :]  # [T, 32]
    p = np.arange(128) % half
    cosr = np.cos(ang)[:, p].T.astype(BF)  # [128, T]
    sinr = np.sin(ang)[:, p].T.astype(BF)
    tk = np.arange(128)[:, None]
    tq = np.arange(128)[None, :]
    cm = (tq >= tk).astype(BF)  # [128, 128] causal triangle

    ow_full = np.ascontiguousarray(o_w.T.astype(BF))  # [D(f), D(d_out)]

    in_maps = []
    for c in range(8):
        b, r = c // 4, c % 4
        qs = slice(r * VF, (r + 1) * VF)
        qr = qkv_w[0 * D :][qs] * n1_w[None, :]
        kr = qkv_w[1 * D :][qs] * n1_w[None, :]
        vr = qkv_w[2 * D :][qs] * n1_w[None, :]
        ms = slice(r * MID, (r + 1) * MID)
        in_maps.append(
            {
                "x": np.ascontiguousarray(x[b, :T], np.float32),
                "qkw_t": np.ascontiguousarray(
                    np.concatenate([qr, kr], 0).T.astype(BF)
                ),
                "vw_m": np.ascontiguousarray(vr.T.astype(BF)),
                "ow_m": ow_full,
                "w1w_t": np.ascontiguousarray(
                    (w1_w[ms] * n2_w[None, :]).T.astype(BF)
                ),
                "wgw_t": np.ascontiguousarray(
                    (wg_w[ms] * n2_w[None, :]).T.astype(BF)
                ),
                "w2w_m": np.ascontiguousarray(w2_w[:, ms].T.astype(BF)),
                "cosr": cosr,
                "sinr": sinr,
                "cmask": cm,
                "ident": np.eye(128, dtype=BF),
            }
        )
    return in_maps


_CACHE = {}


def _get_nc(T):
    if T not in _CACHE:
        _CACHE[T] = build_nc(T, use_silu=True)
    return _CACHE[T]


def run(inputs, T=2048, trace=False):
    nc = _get_nc(T)
    in_maps = make_in_maps(T=T, **inputs)
    res = run_bass_kernel_spmd(nc, in_maps, core_ids=list(range(8)), trace=trace)
    QT = min(512, T)
    NQ = T // QT
    out = np.empty((B, T, D), dtype=np.float32)
    for b in range(B):
        for r in range(TP):
            shard = res.results[b * TP + r]["out"]  # [NQ*128, D]
            for k in range(NQ):
                # each chunk was reduce-scattered in two halves
                for h in range(2):
                    lo = k * QT + h * 256 + r * 64
                    out[b, lo : lo + 64] = shard[
                        k * 128 + h * 64 : k * 128 + (h + 1) * 64
                    ]
    return out, res


def kernel(**inputs):
    out, _ = run(inputs, T=2048)
    return out


# revision 62
# speedup vs baseline: 1.0094x; 1.0094x over previous
"""Distributed Trainium2 kernel for a dense transformer block.

Reference computation (per batch):
  x = x + o_proj(attn(rope(qkv(rmsnorm(x))), causal)) ; x = x + w2(silu(wg(rmsnorm(x))) * w1(rmsnorm(x)))

Sharding: DP=2 on batch x TP=4 on heads / MLP rows (Megatron).
Cores 0-3 handle batch 0, cores 4-7 batch 1. Within a group, rank r owns
heads 4r..4r+3 and MLP rows 1024r..1024(r+1).

Collective strategy (v2): instead of two AllReduces, use
  - AllGather of the normalized per-head attention outputs (feature-major,
    256KB per rank per 512-token chunk); every rank then computes the FULL
    o-proj locally.  Halves the wire bytes and runs in one mesh phase.
  - ReduceScatter of the MLP output + h/4 residual; each rank ends up with
    a 128-token slice per chunk and Python reassembles the full output from
    all 8 cores.
Both are chunked into 4 pieces and software-pipelined against compute.
"""

import sys

sys.path.insert(0, "/opt/trn_rl_repo")

import numpy as np
import ml_dtypes

import concourse.bass as bass
import concourse.bacc as bacc
import concourse.mybir as mybir
import concourse.tile as tile
from concourse.bass_utils import run_bass_kernel_spmd

BF = ml_dtypes.bfloat16
F32 = mybir.dt.float32
BF16 = mybir.dt.bfloat16

D = 1024
NH = 16
DH = 64
MULT = 4
EPS = 1e-5
ROPE_BASE = 10000.0
B = 2
TP = 4  # tensor-parallel ranks per group
HPC = NH // TP  # heads per core = 4
QKF = 2 * HPC * DH  # q+k shard features = 512
VF = HPC * DH  # v shard features = 256
MID = MULT * D // TP  # mlp rows per core = 1024
AF = mybir.ActivationFunctionType
ALU = mybir.AluOpType


def build_nc(T, use_silu=False):
    """Build the SPMD graph for one core (token count T per batch)."""
    DC = D // 128  # d chunks = 8
    TT = T // 128  # token tiles
    QT = min(512, T)  # q-tile width == collective chunk width
    NQ = T // QT
    CPQ = QT // 128  # 128-token tiles per chunk
    MIDC = MID // 128  # mlp row chunks = 8
    NT = D // 512

    nc = bacc.Bacc("TRN2", target_bir_lowering=False, debug=False, num_devices=8)

    x_e = nc.dram_tensor("x", [T, D], F32, kind="ExternalInput")
    qkw_e = nc.dram_tensor("qkw_t", [D, QKF], BF16, kind="ExternalInput")
    vw_e = nc.dram_tensor("vw_m", [D, VF], BF16, kind="ExternalInput")
    ow_e = nc.dram_tensor("ow_m", [D, D], BF16, kind="ExternalInput")
    w1w_e = nc.dram_tensor("w1w_t", [D, MID], BF16, kind="ExternalInput")
    wgw_e = nc.dram_tensor("wgw_t", [D, MID], BF16, kind="ExternalInput")
    w2w_e = nc.dram_tensor("w2w_m", [MID, D], BF16, kind="ExternalInput")
    cos_e = nc.dram_tensor("cosr", [128, T], BF16, kind="ExternalInput")
    sin_e = nc.dram_tensor("sinr", [128, T], BF16, kind="ExternalInput")
    cm_e = nc.dram_tensor("cmask", [128, 128], BF16, kind="ExternalInput")
    id_e = nc.dram_tensor("ident", [128, 128], BF16, kind="ExternalInput")
    out_e = nc.dram_tensor("out", [NQ * 128, D], F32, kind="ExternalOutput")

    groups = [[0, 1, 2, 3], [4, 5, 6, 7]]

    with tile.TileContext(nc) as tc:
        with (
            tc.tile_pool(name="const", bufs=1) as cpool,
            tc.tile_pool(name="actfm", bufs=1) as fmpool,
            tc.tile_pool(name="qko", bufs=1) as qkpool,
            tc.tile_pool(name="vaug", bufs=1) as vpool,
            tc.tile_pool(name="xin", bufs=4) as xpool,
            tc.tile_pool(name="xnb", bufs=3) as xnpool,
            tc.tile_pool(name="work", bufs=4) as wpool,
            tc.tile_pool(name="rope", bufs=2) as rpool,
            tc.tile_pool(name="stats", bufs=8) as spool,
            tc.tile_pool(name="hres", bufs=8) as hpool,
            tc.tile_pool(name="agos", bufs=2) as agpool,
            tc.tile_pool(name="psA", bufs=4, space="PSUM") as psA,
            tc.tile_pool(name="psO", bufs=2, space="PSUM") as psO,
            tc.tile_pool(name="psS", bufs=2, space="PSUM") as psS,
            tc.tile_pool(name="dram", bufs=1, space="DRAM") as dpool,
        ):
            # ---- resident weights / tables ----
            def load_tiles(src, width, n, dt=BF16):
                ts = []
                for i in range(n):
                    t = cpool.tile(
                        [128, width], dt, tag=f"{src.name}_{i}", name=f"{src.name}_{i}"
                    )
                    nc.sync.dma_start(t[:], src[i * 128 : (i + 1) * 128, :])
                    ts.append(t)
                return ts

            # ident + epsc + chunk-0 x tiles first: the norm1 chain for
            # chunk 0 doesn't wait behind the weight stack's DMAs.
            ident = load_tiles(id_e, 128, 1)[0]
            epsc = cpool.tile([128, 1], F32, tag="epsc", name="epsc")
            nc.vector.memset(epsc[:], EPS)
            x0_tiles = []
            for tau in range(CPQ):
                xt = xpool.tile([128, D], F32, tag="xt", name="xt")
                nc.sync.dma_start(xt[:], x_e[tau * 128 : (tau + 1) * 128, :])
                x0_tiles.append(xt)

            qkw = load_tiles(qkw_e, QKF, DC)
            vw = load_tiles(vw_e, VF, DC)
            ow = load_tiles(ow_e, D, DC)
            w1r = w1w_e.rearrange("(c p) m -> p c m", p=128)
            wgr = wgw_e.rearrange("(c p) m -> p c m", p=128)
            cos_t, sin_t = [], []
            for t4 in range(NQ):
                for src, dst in ((cos_e, cos_t), (sin_e, sin_t)):
                    t = cpool.tile(
                        [128, QT], BF16, tag=f"{src.name}_{t4}",
                        name=f"{src.name}c{t4}",
                    )
                    nc.sync.dma_start(t[:], src[:, t4 * QT : (t4 + 1) * QT])
                    dst.append(t)
            tri = load_tiles(cm_e, 128, 1)[0]
            ones64 = cpool.tile([1, 64], BF16, tag="ones64", name="ones64")
            nc.vector.memset(ones64[:], 1.0)

            ag_in = [
                dpool.tile([VF, QT], BF16, name=f"ag_in{k}") for k in range(NQ)
            ]
            ag_out = [
                dpool.tile([TP, VF, QT], BF16, name=f"ag_out{k}")
                for k in range(NQ)
            ]
            rs_in = [
                dpool.tile([QT, D], BF16, name=f"rs_in{k}") for k in range(NQ)
            ]
            rs_out = [
                dpool.tile([QT // TP, D], BF16, name=f"rs_out{k}")
                for k in range(NQ)
            ]

            # ---- persistent activation tiles ----
            # chunk-local normalized activations, feature-major:
            # fm[:, dc, tau*128:...] holds (x-hat chunk)^T for d-block dc
            xnf_c = [
                fmpool.tile(
                    [128, DC, QT], BF16, tag="fm", name=f"xnf{t4}", bufs=2
                )
                for t4 in range(NQ)
            ]
            q_sb = [
                qkpool.tile([128, T], BF16, tag=f"qk{i}", name=f"q{i}")
                for i in range(2)
            ]
            k_sb = [
                qkpool.tile([128, T], BF16, tag=f"qk{i + 2}", name=f"k{i}")
                for i in range(2)
            ]
            O_sb = [
                qkpool.tile([128, T], BF16, tag=f"qk{i + 4}", name=f"O{i}")
                for i in range(2)
            ]
            On_sb = [
                qkpool.tile([128, T], BF16, tag=f"qk{i + 6}", name=f"On{i}")
                for i in range(2)
            ]
            v_aug = [
                vpool.tile([128, HPC, DH + 1], BF16, tag=f"va{ti}", name=f"va{ti}")
                for ti in range(TT)
            ]

            # ---- helpers ----
            def norm_into_fm(xt, fm_c, tau):
                """rmsnorm the token tile xt, write bf16 feature-major via
                a single DMA-transpose into fm_c[:, :, tau*128:(tau+1)*128]."""
                ss = spool.tile([128, 1], F32, tag="ss", name="ss")
                sq = xnpool.tile([128, D], BF16, tag="sq", name="sq", bufs=1)
                nc.vector.scalar_tensor_tensor(
                    sq[:], xt[:], 1.0, xt[:], ALU.mult, ALU.mult, accum_out=ss[:]
                )
                sr = spool.tile([128, 1], F32, tag="sr", name="sr")
                nc.scalar.activation(
                    out=sr[:], in_=ss[:], func=AF.Sqrt, bias=epsc[:], scale=1.0 / D
                )
                s1 = spool.tile([128, 1], F32, tag="s1", name="s1")
                nc.vector.reciprocal(s1[:], sr[:])
                xn = xnpool.tile([128, D], BF16, tag="xn", name="xn")
                nc.vector.tensor_scalar_mul(xn[:], xt[:], s1[:])
                for di in range(DC):
                    tp = psS.tile([128, 128], BF16, tag="tp", name="tp", bufs=2)
                    nc.tensor.transpose(
                        tp[:], xn[:, di * 128 : (di + 1) * 128], ident[:]
                    )
                    nc.any.tensor_copy(
                        fm_c[:, di, tau * 128 : (tau + 1) * 128], tp[:]
                    )

            # ---- stage A/B interleaved: norm1 + qkv + rope per chunk ----
            def norm1_t4(t4):
                for tau in range(CPQ):
                    ti = t4 * CPQ + tau
                    if t4 == 0:
                        xt = x0_tiles[tau]
                    else:
                        xt = xpool.tile([128, D], F32, tag="xt", name="xt")
                        nc.sync.dma_start(
                            xt[:], x_e[ti * 128 : (ti + 1) * 128, :]
                        )
                    norm_into_fm(xt, xnf_c[t4], tau)

            def qk_t4(t4):
                tsl = slice(t4 * QT, (t4 + 1) * QT)
                for m in range(4):  # q01 q23 k01 k23
                    dst = q_sb[m] if m < 2 else k_sb[m - 2]
                    ps = psA.tile(
                        [128, 512 if T >= 512 else T], F32, tag="ps", name="ps"
                    )
                    for dc in range(DC):
                        nc.tensor.matmul(
                            ps[:, :QT],
                            qkw[dc][:, m * 128 : (m + 1) * 128],
                            xnf_c[t4][:, dc, :],
                            start=(dc == 0),
                            stop=(dc == DC - 1),
                        )
                    qb = rpool.tile([128, QT], BF16, tag="qb", name="qb")
                    nc.scalar.copy(qb[:], ps[:, :QT])
                    rot = rpool.tile([128, QT], BF16, tag="rot", name="rot")
                    for hb in (0, 64):
                        nc.vector.tensor_scalar_mul(
                            rot[hb : hb + 32, :], qb[hb + 32 : hb + 64, :], -1.0
                        )
                        nc.vector.tensor_copy(
                            rot[hb + 32 : hb + 64, :], qb[hb : hb + 32, :]
                        )
                    t1 = rpool.tile([128, QT], BF16, tag="t1", name="t1")
                    nc.vector.tensor_mul(t1[:], qb[:], cos_t[t4][:])
                    t2 = rpool.tile([128, QT], BF16, tag="t2", name="t2")
                    nc.vector.tensor_mul(t2[:], rot[:], sin_t[t4][:])
                    nc.vector.tensor_add(dst[:, tsl], t1[:], t2[:])

            def v_chunk(t4):
                for tau in range(CPQ):
                    ti = t4 * CPQ + tau
                    ps = psS.tile([128, VF], F32, tag="tp", name="psv")
                    for dc in range(DC):
                        nc.tensor.matmul(
                            ps[:],
                            xnf_c[t4][:, dc, tau * 128 : (tau + 1) * 128],
                            vw[dc][:],
                            start=(dc == 0),
                            stop=(dc == DC - 1),
                        )
                    va = v_aug[ti]
                    nc.vector.tensor_copy(
                        va[:, :, 0:DH], ps.rearrange("p (h d) -> p h d", h=HPC)
                    )
                    nc.vector.memset(va[:, :, DH : DH + 1], 1.0)

            # ---- stage C: attention + AllGather of normalized head outputs ----
            def attn_qtile(qt):
                tsl = slice(qt * QT, (qt + 1) * QT)
                ncks = CPQ * (qt + 1)
                dnb = spool.tile(
                    [1, HPC * QT], BF16, tag="dnb", name="dnb", bufs=1
                )
                for hp in range(2):
                    opsP = [
                        psO.tile([DH + 1, QT], F32, tag="pso", name=f"ops{i}")
                        for i in range(2)
                    ]

                    def emit_scores(ck):
                        j = ck - CPQ * qt  # >=0 inside the diagonal block
                        lo = max(j, 0) * 128
                        pts = []
                        for i in range(2):
                            hb = i * 64
                            sp = psA.tile(
                                [128, 512 if T >= 512 else T], F32, tag="ps", name="sp"
                            )
                            nc.tensor.matmul(
                                sp[:, :QT],
                                k_sb[hp][hb : hb + DH, ck * 128 : (ck + 1) * 128],
                                q_sb[hp][hb : hb + DH, tsl],
                                start=True,
                                stop=True,
                            )
                            pt = wpool.tile(
                                [128, QT], BF16, tag="pt", name="pt", bufs=6
                            )
                            if j > 0:
                                nc.vector.memset(pt[:, :lo], 0.0)
                            nc.scalar.activation(
                                out=pt[:, lo:],
                                in_=sp[:, lo:QT],
                                func=AF.Exp,
                                scale=0.125,
                            )
                            if j >= 0:
                                nc.vector.tensor_mul(
                                    pt[:, lo : lo + 128], pt[:, lo : lo + 128], tri[:]
                                )
                            pts.append(pt)
                        return pts

                    def emit_av(ck, pts):
                        for i in range(2):
                            nc.tensor.matmul(
                                opsP[i][:],
                                v_aug[ck][:, 2 * hp + i, :],
                                pts[i][:],
                                start=(ck == 0),
                                stop=(ck == ncks - 1),
                            )

                    # scores emitted one chunk ahead of AV so the PE never
                    # waits on the Exp chain
                    prev = emit_scores(0)
                    for ck in range(1, ncks):
                        cur = emit_scores(ck)
                        emit_av(ck - 1, prev)
                        prev = cur
                    emit_av(ncks - 1, prev)
                    for i in range(2):
                        h = 2 * hp + i
                        ops = opsP[i]
                        nc.vector.tensor_copy(
                            dnb[:, h * QT : (h + 1) * QT], ops[DH : DH + 1, :]
                        )
                        nc.scalar.copy(
                            O_sb[hp][i * 64 : i * 64 + DH, tsl], ops[0:DH, :]
                        )
                return dnb

            def normalize_qt(qt, dnb):
                tsl = slice(qt * QT, (qt + 1) * QT)
                for ot in range(2):
                    bb = psA.tile(
                        [128, 512 if T >= 512 else T], F32, tag="ps", name="bb"
                    )
                    for i in range(2):
                        h = 2 * ot + i
                        nc.tensor.matmul(
                            bb[i * 64 : (i + 1) * 64, :QT],
                            ones64[:],
                            dnb[:, h * QT : (h + 1) * QT],
                            start=True,
                            stop=True,
                        )
                    rb = wpool.tile([128, QT], BF16, tag="rb", name="rb", bufs=2)
                    with nc.allow_low_precision(
                        reason="softmax denom ~O(1); bf16 recip matches prior"
                    ):
                        nc.vector.reciprocal(rb[:], bb[:, :QT])
                    nc.vector.tensor_mul(
                        On_sb[ot][:, tsl], O_sb[ot][:, tsl], rb[:]
                    )

            def ag_fire(qt):
                tsl = slice(qt * QT, (qt + 1) * QT)
                for ot in range(2):
                    nc.sync.dma_start(
                        ag_in[qt][ot * 128 : (ot + 1) * 128, :], On_sb[ot][:, tsl]
                    )
                nc.gpsimd.collective_compute(
                    "AllGather",
                    ALU.bypass,
                    ins=[ag_in[qt][:].opt()],
                    outs=[ag_out[qt][:].opt()],
                    replica_groups=groups,
                )

            # ---- stage D: full o-proj + residual + norm2 (per chunk) ----
            hnf_c = [
                fmpool.tile(
                    [128, DC, QT], BF16, tag="fm", name=f"hnf{k}", bufs=2
                )
                for k in range(NQ)
            ]
            hres = [
                hpool.tile([128, D], BF16, tag="hr", name=f"hr{ti}", bufs=8)
                for ti in range(TT)
            ]

            def ago_load(k):
                # gathered normalized head outputs, feature-major [1024, QT]
                ago = agpool.tile(
                    [128, TP, 2, QT], BF16, tag="ago", name=f"ago{k}", bufs=2
                )
                agr = ag_out[k][:].rearrange("r (c p) t -> p r c t", p=128)
                nc.sync.dma_start(ago[:], agr)
                return ago

            def oproj_chunk(k, ago):
                for tau in range(CPQ):
                    ti = k * CPQ + tau
                    xo = xpool.tile([128, D], F32, tag="xo", name="xo")
                    nc.sync.dma_start(xo[:], x_e[ti * 128 : (ti + 1) * 128, :])
                    ob = hres[ti]
                    csl = slice(tau * 128, (tau + 1) * 128)
                    for nt in range(NT):
                        ps = psA.tile([128, 512], F32, tag="ps", name="ps")
                        for c in range(DC):
                            nc.tensor.matmul(
                                ps[:, :512],
                                ago[:, c // 2, c % 2, csl],
                                ow[c][:, nt * 512 : (nt + 1) * 512],
                                start=(c == 0),
                                stop=(c == DC - 1),
                            )
                        nc.vector.scalar_tensor_tensor(
                            ob[:, nt * 512 : (nt + 1) * 512],
                            xo[:, nt * 512 : (nt + 1) * 512],
                            1.0,
                            ps[:, :512],
                            ALU.mult,
                            ALU.add,
                        )
                    norm_into_fm(ob, hnf_c[k], tau)

            # ---- stage E: MLP (chunk-pipelined) + ReduceScatter ----
            # w2 is loaded as 16 half-tiles [128, 512], reusing the SBUF of
            # the (now dead) qkv weights and causal-mask tiles.
            host_tags = (
                [f"qkw_t_{i}" for i in range(DC)]
                + [f"cosr_{i}" for i in range(NQ)]
                + [f"sinr_{i}" for i in range(NQ)]
            )
            w2h = []
            for i in range(2 * MIDC):
                t = cpool.tile(
                    [128, 512], BF16, tag=host_tags[i], name=f"w2h{i}"
                )
                nc.sync.dma_start(
                    t[:],
                    w2w_e[
                        (i // 2) * 128 : (i // 2 + 1) * 128,
                        (i % 2) * 512 : (i % 2 + 1) * 512,
                    ],
                )
                w2h.append(t)
            a_fm = [
                qkpool.tile([128, T], BF16, tag=f"qk{d}", name=f"a{d}")
                for d in range(MIDC)
            ]

            def mlp_t4(t4):
                tsl = slice(t4 * QT, (t4 + 1) * QT)
                for mc in range(MIDC):
                    msl = slice(mc * 128, (mc + 1) * 128)
                    wg_mc = wpool.tile(
                        [128, DC, 128], BF16, tag="wgs", name="wg_mc", bufs=2
                    )
                    nc.sync.dma_start(wg_mc[:], wgr[:, :, msl])
                    w1_mc = wpool.tile(
                        [128, DC, 128], BF16, tag="w1s", name="w1_mc", bufs=2
                    )
                    nc.sync.dma_start(w1_mc[:], w1r[:, :, msl])
                    psg = psA.tile(
                        [128, 512 if T >= 512 else T], F32, tag="ps", name="psg"
                    )
                    for dc in range(DC):
                        nc.tensor.matmul(
                            psg[:, :QT],
                            wg_mc[:, dc, :],
                            hnf_c[t4][:, dc, :],
                            start=(dc == 0),
                            stop=(dc == DC - 1),
                        )
                    g_sb = wpool.tile([128, QT], BF16, tag="g", name="g", bufs=2)
                    if use_silu:
                        nc.scalar.activation(
                            out=g_sb[:], in_=psg[:, :QT], func=AF.Silu
                        )
                    else:
                        sg = wpool.tile([128, QT], F32, tag="sg", name="sg", bufs=2)
                        nc.scalar.activation(
                            out=sg[:], in_=psg[:, :QT], func=AF.Sigmoid
                        )
                        nc.vector.tensor_mul(g_sb[:], sg[:], psg[:, :QT])
                    psu = psA.tile(
                        [128, 512 if T >= 512 else T], F32, tag="ps", name="psu"
                    )
                    for dc in range(DC):
                        nc.tensor.matmul(
                            psu[:, :QT],
                            w1_mc[:, dc, :],
                            hnf_c[t4][:, dc, :],
                            start=(dc == 0),
                            stop=(dc == DC - 1),
                        )
                    nc.vector.tensor_mul(a_fm[mc][:, tsl], g_sb[:], psu[:, :QT])

            def w2_rs(t4):
                for tau in range(CPQ):
                    ti = t4 * CPQ + tau
                    ob = wpool.tile([128, D], BF16, tag="ob", name="ob", bufs=2)
                    for nt in range(NT):
                        ps = psA.tile([128, 512], F32, tag="ps", name="ps")
                        for mc in range(MIDC):
                            nc.tensor.matmul(
                                ps[:, :512],
                                a_fm[mc][:, ti * 128 : (ti + 1) * 128],
                                w2h[2 * mc + nt][:],
                                start=(mc == 0),
                                stop=(mc == MIDC - 1),
                            )
                        nc.vector.scalar_tensor_tensor(
                            ob[:, nt * 512 : (nt + 1) * 512],
                            hres[ti][:, nt * 512 : (nt + 1) * 512],
                            1.0 / TP,
                            ps[:, :512],
                            ALU.mult,
                            ALU.add,
                        )
                    nc.sync.dma_start(
                        rs_in[t4][tau * 128 : (tau + 1) * 128, :],
                        ob[:],
                    )
                    if tau % 2 == 1:
                        # two half-size RS ops per chunk: each fires as soon
                        # as its data is staged, so the serial CC chain never
                        # waits long for local data
                        h = tau // 2
                        nc.gpsimd.collective_compute(
                            "ReduceScatter",
                            ALU.add,
                            ins=[rs_in[t4][h * 256 : (h + 1) * 256, :].opt()],
                            outs=[rs_out[t4][h * 64 : (h + 1) * 64, :].opt()],
                            replica_groups=groups,
                        )

            def final_chunk(k):
                nc.gpsimd.dma_start(
                    out_e[k * 128 : (k + 1) * 128, :],
                    rs_out[k][:],
                )

            # ---- schedule ----
            for t4 in range(NQ):
                norm1_t4(t4)
                qk_t4(t4)
                v_chunk(t4)
            for qt in range(NQ):
                dnb = attn_qtile(qt)
                normalize_qt(qt, dnb)
                ag_fire(qt)
            ago_cur = ago_load(0)
            for k in range(NQ):
                oproj_chunk(k, ago_cur)
                if k + 1 < NQ:
                    ago_cur = ago_load(k + 1)
                mlp_t4(k)
                w2_rs(k)
                if k >= 1:
                    final_chunk(k - 1)
            final_chunk(NQ - 1)

    nc.compile()
    return nc


def make_in_maps(x, n1_w, n2_w, qkv_w, o_w, w1_w, wg_w, w2_w, T):
    QT = min(512, T)
    CPQ = QT // 128
    half = DH // 2
    freqs = np.arange(half, dtype=np.float64) / half
    theta = 1.0 / ROPE_BASE**freqs
    ang = np.arange(T, dtype=np.float64)[:, None] * theta[None, :]  # [T, 32]
    p = np.arange(128) % half
    cosr = np.cos(ang)[:, p].T.astype(BF)  # [128, T]
    sinr = np.sin(ang)[:, p].T.astype(BF)
    tk = np.arange(128)[:, None]
    tq = np.arange(128)[None, :]
    cm = (tq >= tk).astype(BF)  # [128, 128] causal triangle

    ow_full = np.ascontiguousarray(o_w.T.astype(BF))  # [D(f), D(d_out)]

    in_maps = []
    for c in range(8):
        b, r = c // 4, c % 4
        qs = slice(r * VF, (r + 1) * VF)
        qr = qkv_w[0 * D :][qs] * n1_w[None, :]
        kr = qkv_w[1 * D :][qs] * n1_w[None, :]
        vr = qkv_w[2 * D :][qs] * n1_w[None, :]
        ms = slice(r * MID, (r + 1) * MID)
        in_maps.append(
            {
                "x": np.ascontiguousarray(x[b, :T], np.float32),
                "qkw_t": np.ascontiguousarray(
                    np.concatenate([qr, kr], 0).T.astype(BF)
                ),
                "vw_m": np.ascontiguousarray(vr.T.astype(BF)),
                "ow_m": ow_full,
                "w1w_t": np.ascontiguousarray(
                    (w1_w[ms] * n2_w[None, :]).T.astype(BF)
                ),
                "wgw_t": np.ascontiguousarray(
                    (wg_w[ms] * n2_w[None, :]).T.astype(BF)
                ),
                "w2w_m": np.ascontiguousarray(w2_w[:, ms].T.astype(BF)),
                "cosr": cosr,
                "sinr": sinr,
                "cmask": cm,
                "ident": np.eye(128, dtype=BF),
            }
        )
    return in_maps


_CACHE = {}


def _get_nc(T):
    if T not in _CACHE:
        _CACHE[T] = build_nc(T, use_silu=True)
    return _CACHE[T]


def run(inputs, T=2048, trace=False):
    nc = _get_nc(T)
    in_maps = make_in_maps(T=T, **inputs)
    res = run_bass_kernel_spmd(nc, in_maps, core_ids=list(range(8)), trace=trace)
    QT = min(512, T)
    NQ = T // QT
    out = np.empty((B, T, D), dtype=np.float32)
    for b in range(B):
        for r in range(TP):
            shard = res.results[b * TP + r]["out"]  # [NQ*128, D]
            for k in range(NQ):
                # each chunk was reduce-scattered in two halves
                for h in range(2):
                    lo = k * QT + h * 256 + r * 64
                    out[b, lo : lo + 64] = shard[
                        k * 128 + h * 64 : k * 128 + (h + 1) * 64
                    ]
    return out, res


def kernel(**inputs):
    out, _ = run(inputs, T=2048)
    return out


# revision 64
# speedup vs baseline: 1.0288x; 1.0193x over previous
"""Distributed Trainium2 kernel for a dense transformer block.

Reference computation (per batch):
  x = x + o_proj(attn(rope(qkv(rmsnorm(x))), causal)) ; x = x + w2(silu(wg(rmsnorm(x))) * w1(rmsnorm(x)))

Sharding: DP=2 on batch x TP=4 on heads / MLP rows (Megatron).
Cores 0-3 handle batch 0, cores 4-7 batch 1. Within a group, rank r owns
heads 4r..4r+3 and MLP rows 1024r..1024(r+1).

Collective strategy (v2): instead of two AllReduces, use
  - AllGather of the normalized per-head attention outputs (feature-major,
    256KB per rank per 512-token chunk); every rank then computes the FULL
    o-proj locally.  Halves the wire bytes and runs in one mesh phase.
  - ReduceScatter of the MLP output + h/4 residual; each rank ends up with
    a 128-token slice per chunk and Python reassembles the full output from
    all 8 cores.
Both are chunked into 4 pieces and software-pipelined against compute.
"""

import sys

sys.path.insert(0, "/opt/trn_rl_repo")

import numpy as np
import ml_dtypes

import concourse.bass as bass
import concourse.bacc as bacc
import concourse.mybir as mybir
import concourse.tile as tile
from concourse.bass_utils import run_bass_kernel_spmd

BF = ml_dtypes.bfloat16
F32 = mybir.dt.float32
BF16 = mybir.dt.bfloat16

D = 1024
NH = 16
DH = 64
MULT = 4
EPS = 1e-5
ROPE_BASE = 10000.0
B = 2
TP = 4  # tensor-parallel ranks per group
HPC = NH // TP  # heads per core = 4
QKF = 2 * HPC * DH  # q+k shard features = 512
VF = HPC * DH  # v shard features = 256
MID = MULT * D // TP  # mlp rows per core = 1024
AF = mybir.ActivationFunctionType
ALU = mybir.AluOpType


def build_nc(T, use_silu=False):
    """Build the SPMD graph for one core (token count T per batch)."""
    DC = D // 128  # d chunks = 8
    TT = T // 128  # token tiles
    QT = min(512, T)  # q-tile width == collective chunk width
    NQ = T // QT
    CPQ = QT // 128  # 128-token tiles per chunk
    MIDC = MID // 128  # mlp row chunks = 8
    NT = D // 512

    nc = bacc.Bacc("TRN2", target_bir_lowering=False, debug=False, num_devices=8)

    x_e = nc.dram_tensor("x", [T, D], F32, kind="ExternalInput")
    qkw_e = nc.dram_tensor("qkw_t", [D, QKF], BF16, kind="ExternalInput")
    vw_e = nc.dram_tensor("vw_m", [D, VF], BF16, kind="ExternalInput")
    ow_e = nc.dram_tensor("ow_m", [D, D], BF16, kind="ExternalInput")
    w1w_e = nc.dram_tensor("w1w_t", [D, MID], BF16, kind="ExternalInput")
    wgw_e = nc.dram_tensor("wgw_t", [D, MID], BF16, kind="ExternalInput")
    w2w_e = nc.dram_tensor("w2w_m", [MID, D], BF16, kind="ExternalInput")
    cos_e = nc.dram_tensor("cosr", [128, T], BF16, kind="ExternalInput")
    sin_e = nc.dram_tensor("sinr", [128, T], BF16, kind="ExternalInput")
    cm_e = nc.dram_tensor("cmask", [128, 128], BF16, kind="ExternalInput")
    id_e = nc.dram_tensor("ident", [128, 128], BF16, kind="ExternalInput")
    out_e = nc.dram_tensor("out", [NQ * 128, D], F32, kind="ExternalOutput")

    groups = [[0, 1, 2, 3], [4, 5, 6, 7]]

    with tile.TileContext(nc) as tc:
        with (
            tc.tile_pool(name="const", bufs=1) as cpool,
            tc.tile_pool(name="actfm", bufs=1) as fmpool,
            tc.tile_pool(name="qko", bufs=1) as qkpool,
            tc.tile_pool(name="vaug", bufs=1) as vpool,
            tc.tile_pool(name="xin", bufs=4) as xpool,
            tc.tile_pool(name="xnb", bufs=3) as xnpool,
            tc.tile_pool(name="work", bufs=4) as wpool,
            tc.tile_pool(name="rope", bufs=2) as rpool,
            tc.tile_pool(name="stats", bufs=8) as spool,
            tc.tile_pool(name="hres", bufs=8) as hpool,
            tc.tile_pool(name="agos", bufs=2) as agpool,
            tc.tile_pool(name="psA", bufs=4, space="PSUM") as psA,
            tc.tile_pool(name="psO", bufs=2, space="PSUM") as psO,
            tc.tile_pool(name="psS", bufs=2, space="PSUM") as psS,
            tc.tile_pool(name="dram", bufs=1, space="DRAM") as dpool,
        ):
            # ---- resident weights / tables ----
            def load_tiles(src, width, n, dt=BF16):
                ts = []
                for i in range(n):
                    t = cpool.tile(
                        [128, width], dt, tag=f"{src.name}_{i}", name=f"{src.name}_{i}"
                    )
                    nc.sync.dma_start(t[:], src[i * 128 : (i + 1) * 128, :])
                    ts.append(t)
                return ts

            # ident + epsc + chunk-0 x tiles first: the norm1 chain for
            # chunk 0 doesn't wait behind the weight stack's DMAs.
            ident = load_tiles(id_e, 128, 1)[0]
            epsc = cpool.tile([128, 1], F32, tag="epsc", name="epsc")
            nc.vector.memset(epsc[:], EPS)
            x0_tiles = []
            for tau in range(CPQ):
                xt = xpool.tile([128, D], F32, tag="xt", name="xt")
                nc.sync.dma_start(xt[:], x_e[tau * 128 : (tau + 1) * 128, :])
                x0_tiles.append(xt)

            qkw = load_tiles(qkw_e, QKF, DC)
            vw = load_tiles(vw_e, VF, DC)
            ow = load_tiles(ow_e, D, DC)
            w1r = w1w_e.rearrange("(c p) m -> p c m", p=128)
            wgr = wgw_e.rearrange("(c p) m -> p c m", p=128)
            cos_t, sin_t = [], []
            for t4 in range(NQ):
                for src, dst in ((cos_e, cos_t), (sin_e, sin_t)):
                    t = cpool.tile(
                        [128, QT], BF16, tag=f"{src.name}_{t4}",
                        name=f"{src.name}c{t4}",
                    )
                    nc.sync.dma_start(t[:], src[:, t4 * QT : (t4 + 1) * QT])
                    dst.append(t)
            tri = load_tiles(cm_e, 128, 1)[0]
            ones64 = cpool.tile([1, 64], BF16, tag="ones64", name="ones64")
            nc.vector.memset(ones64[:], 1.0)

            ag_in = [
                dpool.tile([VF, QT], BF16, name=f"ag_in{k}") for k in range(NQ)
            ]
            ag_out = [
                dpool.tile([TP, VF, QT], BF16, name=f"ag_out{k}")
                for k in range(NQ)
            ]
            rs_in = [
                dpool.tile([QT, D], BF16, name=f"rs_in{k}") for k in range(NQ)
            ]
            rs_out = [
                dpool.tile([QT // TP, D], BF16, name=f"rs_out{k}")
                for k in range(NQ)
            ]

            # ---- persistent activation tiles ----
            # chunk-local normalized activations, feature-major:
            # fm[:, dc, tau*128:...] holds (x-hat chunk)^T for d-block dc
            xnf_c = [
                fmpool.tile(
                    [128, DC, QT], BF16, tag="fm", name=f"xnf{t4}", bufs=2
                )
                for t4 in range(NQ)
            ]
            q_sb = [
                qkpool.tile([128, T], BF16, tag=f"qk{i}", name=f"q{i}")
                for i in range(2)
            ]
            k_sb = [
                qkpool.tile([128, T], BF16, tag=f"qk{i + 2}", name=f"k{i}")
                for i in range(2)
            ]
            O_sb = [
                qkpool.tile([128, T], BF16, tag=f"qk{i + 4}", name=f"O{i}")
                for i in range(2)
            ]
            On_sb = [
                qkpool.tile([128, T], BF16, tag=f"qk{i + 6}", name=f"On{i}")
                for i in range(2)
            ]
            v_aug = [
                vpool.tile([128, HPC, DH + 1], BF16, tag=f"va{ti}", name=f"va{ti}")
                for ti in range(TT)
            ]

            # ---- helpers ----
            def norm_into_fm(xt, fm_c, tau):
                """rmsnorm the token tile xt, write bf16 feature-major via
                a single DMA-transpose into fm_c[:, :, tau*128:(tau+1)*128]."""
                ss = spool.tile([128, 1], F32, tag="ss", name="ss")
                sq = xnpool.tile([128, D], BF16, tag="sq", name="sq", bufs=1)
                nc.vector.scalar_tensor_tensor(
                    sq[:], xt[:], 1.0, xt[:], ALU.mult, ALU.mult, accum_out=ss[:]
                )
                sr = spool.tile([128, 1], F32, tag="sr", name="sr")
                nc.scalar.activation(
                    out=sr[:], in_=ss[:], func=AF.Sqrt, bias=epsc[:], scale=1.0 / D
                )
                s1 = spool.tile([128, 1], F32, tag="s1", name="s1")
                nc.vector.reciprocal(s1[:], sr[:])
                xn = xnpool.tile([128, D], BF16, tag="xn", name="xn")
                nc.vector.tensor_scalar_mul(xn[:], xt[:], s1[:])
                for di in range(DC):
                    tp = psS.tile([128, 128], BF16, tag="tp", name="tp", bufs=2)
                    nc.tensor.transpose(
                        tp[:], xn[:, di * 128 : (di + 1) * 128], ident[:]
                    )
                    nc.any.tensor_copy(
                        fm_c[:, di, tau * 128 : (tau + 1) * 128], tp[:]
                    )

            # ---- stage A/B interleaved: norm1 + qkv + rope per chunk ----
            def norm1_t4(t4):
                for tau in range(CPQ):
                    ti = t4 * CPQ + tau
                    if t4 == 0:
                        xt = x0_tiles[tau]
                    else:
                        xt = xpool.tile([128, D], F32, tag="xt", name="xt")
                        nc.sync.dma_start(
                            xt[:], x_e[ti * 128 : (ti + 1) * 128, :]
                        )
                    norm_into_fm(xt, xnf_c[t4], tau)

            def qk_t4(t4):
                tsl = slice(t4 * QT, (t4 + 1) * QT)
                for m in range(4):  # q01 q23 k01 k23
                    dst = q_sb[m] if m < 2 else k_sb[m - 2]
                    ps = psA.tile(
                        [128, 512 if T >= 512 else T], F32, tag="ps", name="ps"
                    )
                    for dc in range(DC):
                        nc.tensor.matmul(
                            ps[:, :QT],
                            qkw[dc][:, m * 128 : (m + 1) * 128],
                            xnf_c[t4][:, dc, :],
                            start=(dc == 0),
                            stop=(dc == DC - 1),
                        )
                    qb = rpool.tile([128, QT], BF16, tag="qb", name="qb")
                    nc.scalar.copy(qb[:], ps[:, :QT])
                    rot = rpool.tile([128, QT], BF16, tag="rot", name="rot")
                    for hb in (0, 64):
                        nc.vector.tensor_scalar_mul(
                            rot[hb : hb + 32, :], qb[hb + 32 : hb + 64, :], -1.0
                        )
                        nc.vector.tensor_copy(
                            rot[hb + 32 : hb + 64, :], qb[hb : hb + 32, :]
                        )
                    t1 = rpool.tile([128, QT], BF16, tag="t1", name="t1")
                    nc.vector.tensor_mul(t1[:], qb[:], cos_t[t4][:])
                    t2 = rpool.tile([128, QT], BF16, tag="t2", name="t2")
                    nc.vector.tensor_mul(t2[:], rot[:], sin_t[t4][:])
                    nc.vector.tensor_add(dst[:, tsl], t1[:], t2[:])

            def v_chunk(t4):
                for tau in range(CPQ):
                    ti = t4 * CPQ + tau
                    ps = psS.tile([128, VF], F32, tag="tp", name="psv")
                    for dc in range(DC):
                        nc.tensor.matmul(
                            ps[:],
                            xnf_c[t4][:, dc, tau * 128 : (tau + 1) * 128],
                            vw[dc][:],
                            start=(dc == 0),
                            stop=(dc == DC - 1),
                        )
                    va = v_aug[ti]
                    nc.vector.tensor_copy(
                        va[:, :, 0:DH], ps.rearrange("p (h d) -> p h d", h=HPC)
                    )
                    nc.vector.memset(va[:, :, DH : DH + 1], 1.0)

            # ---- stage C: attention + AllGather of normalized head outputs ----
            def attn_qtile(qt):
                tsl = slice(qt * QT, (qt + 1) * QT)
                ncks = CPQ * (qt + 1)
                dnb = spool.tile(
                    [1, HPC * QT], BF16, tag="dnb", name="dnb", bufs=1
                )
                for hp in range(2):
                    opsP = [
                        psO.tile([DH + 1, QT], F32, tag="pso", name=f"ops{i}")
                        for i in range(2)
                    ]

                    def emit_scores(ck):
                        j = ck - CPQ * qt  # >=0 inside the diagonal block
                        lo = max(j, 0) * 128
                        pts = []
                        for i in range(2):
                            hb = i * 64
                            sp = psA.tile(
                                [128, 512 if T >= 512 else T], F32, tag="ps", name="sp"
                            )
                            nc.tensor.matmul(
                                sp[:, :QT],
                                k_sb[hp][hb : hb + DH, ck * 128 : (ck + 1) * 128],
                                q_sb[hp][hb : hb + DH, tsl],
                                start=True,
                                stop=True,
                            )
                            pt = wpool.tile(
                                [128, QT], BF16, tag="pt", name="pt", bufs=6
                            )
                            if j > 0:
                                nc.vector.memset(pt[:, :lo], 0.0)
                            nc.scalar.activation(
                                out=pt[:, lo:],
                                in_=sp[:, lo:QT],
                                func=AF.Exp,
                                scale=0.125,
                            )
                            if j >= 0:
                                nc.vector.tensor_mul(
                                    pt[:, lo : lo + 128], pt[:, lo : lo + 128], tri[:]
                                )
                            pts.append(pt)
                        return pts

                    def emit_av(ck, pts):
                        for i in range(2):
                            nc.tensor.matmul(
                                opsP[i][:],
                                v_aug[ck][:, 2 * hp + i, :],
                                pts[i][:],
                                start=(ck == 0),
                                stop=(ck == ncks - 1),
                            )

                    # scores emitted one chunk ahead of AV so the PE never
                    # waits on the Exp chain
                    prev = emit_scores(0)
                    for ck in range(1, ncks):
                        cur = emit_scores(ck)
                        emit_av(ck - 1, prev)
                        prev = cur
                    emit_av(ncks - 1, prev)
                    for i in range(2):
                        h = 2 * hp + i
                        ops = opsP[i]
                        nc.vector.tensor_copy(
                            dnb[:, h * QT : (h + 1) * QT], ops[DH : DH + 1, :]
                        )
                        nc.scalar.copy(
                            O_sb[hp][i * 64 : i * 64 + DH, tsl], ops[0:DH, :]
                        )
                return dnb

            def normalize_qt(qt, dnb):
                tsl = slice(qt * QT, (qt + 1) * QT)
                for ot in range(2):
                    bb = psA.tile(
                        [128, 512 if T >= 512 else T], F32, tag="ps", name="bb"
                    )
                    for i in range(2):
                        h = 2 * ot + i
                        nc.tensor.matmul(
                            bb[i * 64 : (i + 1) * 64, :QT],
                            ones64[:],
                            dnb[:, h * QT : (h + 1) * QT],
                            start=True,
                            stop=True,
                        )
                    rb = wpool.tile([128, QT], BF16, tag="rb", name="rb", bufs=2)
                    with nc.allow_low_precision(
                        reason="softmax denom ~O(1); bf16 recip matches prior"
                    ):
                        nc.vector.reciprocal(rb[:], bb[:, :QT])
                    nc.vector.tensor_mul(
                        On_sb[ot][:, tsl], O_sb[ot][:, tsl], rb[:]
                    )

            def ag_fire(qt):
                tsl = slice(qt * QT, (qt + 1) * QT)
                for ot in range(2):
                    nc.sync.dma_start(
                        ag_in[qt][ot * 128 : (ot + 1) * 128, :], On_sb[ot][:, tsl]
                    )
                nc.gpsimd.collective_compute(
                    "AllGather",
                    ALU.bypass,
                    ins=[ag_in[qt][:].opt()],
                    outs=[ag_out[qt][:].opt()],
                    replica_groups=groups,
                )

            # ---- stage D: full o-proj + residual + norm2 (per chunk) ----
            hnf_c = [
                fmpool.tile(
                    [128, DC, QT], BF16, tag="fm", name=f"hnf{k}", bufs=2
                )
                for k in range(NQ)
            ]
            hres = [
                hpool.tile([128, D], BF16, tag="hr", name=f"hr{ti}", bufs=8)
                for ti in range(TT)
            ]

            def ago_load(k):
                # gathered normalized head outputs, feature-major [1024, QT]
                ago = agpool.tile(
                    [128, TP, 2, QT], BF16, tag="ago", name=f"ago{k}", bufs=2
                )
                agr = ag_out[k][:].rearrange("r (c p) t -> p r c t", p=128)
                nc.sync.dma_start(ago[:], agr)
                return ago

            def oproj_chunk(k, ago):
                for tau in range(CPQ):
                    ti = k * CPQ + tau
                    xo = xpool.tile([128, D], F32, tag="xo", name="xo")
                    nc.sync.dma_start(xo[:], x_e[ti * 128 : (ti + 1) * 128, :])
                    ob = hres[ti]
                    csl = slice(tau * 128, (tau + 1) * 128)
                    for nt in range(NT):
                        ps = psA.tile([128, 512], F32, tag="ps", name="ps")
                        for c in range(DC):
                            nc.tensor.matmul(
                                ps[:, :512],
                                ago[:, c // 2, c % 2, csl],
                                ow[c][:, nt * 512 : (nt + 1) * 512],
                                start=(c == 0),
                                stop=(c == DC - 1),
                            )
                        nc.vector.scalar_tensor_tensor(
                            ob[:, nt * 512 : (nt + 1) * 512],
                            xo[:, nt * 512 : (nt + 1) * 512],
                            1.0,
                            ps[:, :512],
                            ALU.mult,
                            ALU.add,
                        )
                    norm_into_fm(ob, hnf_c[k], tau)

            # ---- stage E: MLP (chunk-pipelined) + ReduceScatter ----
            # w2 is loaded as 16 half-tiles [128, 512], reusing the SBUF of
            # the (now dead) qkv weights and causal-mask tiles.
            host_tags = (
                [f"qkw_t_{i}" for i in range(DC)]
                + [f"cosr_{i}" for i in range(NQ)]
                + [f"sinr_{i}" for i in range(NQ)]
            )
            w2h = []
            for i in range(2 * MIDC):
                t = cpool.tile(
                    [128, 512], BF16, tag=host_tags[i], name=f"w2h{i}"
                )
                nc.sync.dma_start(
                    t[:],
                    w2w_e[
                        (i // 2) * 128 : (i // 2 + 1) * 128,
                        (i % 2) * 512 : (i % 2 + 1) * 512,
                    ],
                )
                w2h.append(t)
            a_fm = [
                qkpool.tile([128, T], BF16, tag=f"qk{d}", name=f"a{d}")
                for d in range(MIDC)
            ]

            def mlp_pair(ka, kb):
                # two chunks per weight pass: each stationary wg/w1 slice is
                # loaded once and immediately reused for both chunks
                for mc in range(MIDC):
                    msl = slice(mc * 128, (mc + 1) * 128)
                    wg_mc = wpool.tile(
                        [128, DC, 128], BF16, tag="wgs", name="wg_mc", bufs=2
                    )
                    nc.sync.dma_start(wg_mc[:], wgr[:, :, msl])
                    w1_mc = wpool.tile(
                        [128, DC, 128], BF16, tag="w1s", name="w1_mc", bufs=2
                    )
                    nc.sync.dma_start(w1_mc[:], w1r[:, :, msl])
                    psgs = [
                        psA.tile(
                            [128, 512 if T >= 512 else T], F32, tag="ps",
                            name=f"psg{i}",
                        )
                        for i in range(2)
                    ]
                    for dc in range(DC):
                        for i, t4 in enumerate((ka, kb)):
                            nc.tensor.matmul(
                                psgs[i][:, :QT],
                                wg_mc[:, dc, :],
                                hnf_c[t4][:, dc, :],
                                start=(dc == 0),
                                stop=(dc == DC - 1),
                            )
                    g_sbs = []
                    for i in range(2):
                        g_sb = wpool.tile(
                            [128, QT], BF16, tag=f"g{i}", name=f"g{i}", bufs=2
                        )
                        nc.scalar.activation(
                            out=g_sb[:], in_=psgs[i][:, :QT], func=AF.Silu
                        )
                        g_sbs.append(g_sb)
                    psus = [
                        psA.tile(
                            [128, 512 if T >= 512 else T], F32, tag="ps",
                            name=f"psu{i}",
                        )
                        for i in range(2)
                    ]
                    for dc in range(DC):
                        for i, t4 in enumerate((ka, kb)):
                            nc.tensor.matmul(
                                psus[i][:, :QT],
                                w1_mc[:, dc, :],
                                hnf_c[t4][:, dc, :],
                                start=(dc == 0),
                                stop=(dc == DC - 1),
                            )
                    for i, t4 in enumerate((ka, kb)):
                        tsl = slice(t4 * QT, (t4 + 1) * QT)
                        nc.vector.tensor_mul(
                            a_fm[mc][:, tsl], g_sbs[i][:], psus[i][:, :QT]
                        )

            def w2_rs(t4):
                for tau in range(CPQ):
                    ti = t4 * CPQ + tau
                    ob = wpool.tile([128, D], BF16, tag="ob", name="ob", bufs=2)
                    for nt in range(NT):
                        ps = psA.tile([128, 512], F32, tag="ps", name="ps")
                        for mc in range(MIDC):
                            nc.tensor.matmul(
                                ps[:, :512],
                                a_fm[mc][:, ti * 128 : (ti + 1) * 128],
                                w2h[2 * mc + nt][:],
                                start=(mc == 0),
                                stop=(mc == MIDC - 1),
                            )
                        nc.vector.scalar_tensor_tensor(
                            ob[:, nt * 512 : (nt + 1) * 512],
                            hres[ti][:, nt * 512 : (nt + 1) * 512],
                            1.0 / TP,
                            ps[:, :512],
                            ALU.mult,
                            ALU.add,
                        )
                    nc.sync.dma_start(
                        rs_in[t4][tau * 128 : (tau + 1) * 128, :],
                        ob[:],
                    )
                    if tau % 2 == 1:
                        # two half-size RS ops per chunk: each fires as soon
                        # as its data is staged, so the serial CC chain never
                        # waits long for local data
                        h = tau // 2
                        nc.gpsimd.collective_compute(
                            "ReduceScatter",
                            ALU.add,
                            ins=[rs_in[t4][h * 256 : (h + 1) * 256, :].opt()],
                            outs=[rs_out[t4][h * 64 : (h + 1) * 64, :].opt()],
                            replica_groups=groups,
                        )

            def final_chunk(k):
                nc.gpsimd.dma_start(
                    out_e[k * 128 : (k + 1) * 128, :],
                    rs_out[k][:],
                )

            # ---- schedule ----
            for t4 in range(NQ):
                norm1_t4(t4)
                qk_t4(t4)
                v_chunk(t4)
            for qt in range(NQ):
                dnb = attn_qtile(qt)
                normalize_qt(qt, dnb)
                ag_fire(qt)
            a0 = ago_load(0)
            a1 = ago_load(1)
            oproj_chunk(0, a0)
            oproj_chunk(1, a1)
            a2 = ago_load(2)
            mlp_pair(0, 1)
            w2_rs(0)
            a3 = ago_load(3)
            w2_rs(1)
            final_chunk(0)
            oproj_chunk(2, a2)
            oproj_chunk(3, a3)
            mlp_pair(2, 3)
            w2_rs(2)
            final_chunk(1)
            w2_rs(3)
            final_chunk(2)
            final_chunk(3)

    nc.compile()
    return nc


def make_in_maps(x, n1_w, n2_w, qkv_w, o_w, w1_w, wg_w, w2_w, T):
    QT = min(512, T)
    CPQ = QT // 128
    half = DH // 2
    freqs = np.arange(half, dtype=np.float64) / half
    theta = 1.0 / ROPE_BASE**freqs
    ang = np.arange(T, dtype=np.float64)[:, None] * theta[None, :]  # [T, 32]
    p = np.arange(128) % half
    cosr = np.cos(ang)[:, p].T.astype(BF)  # [128, T]
    sinr = np.sin(ang)[:, p].T.astype(BF)
    tk = np.arange(128)[:, None]
    tq = np.arange(128)[None, :]
    cm = (tq >= tk).astype(BF)  # [128, 128] causal triangle

    ow_full = np.ascontiguousarray(o_w.T.astype(BF))  # [D(f), D(d_out)]

    in_maps = []
    for c in range(8):
        b, r = c // 4, c % 4
        qs = slice(r * VF, (r + 1) * VF)
        qr = qkv_w[0 * D :][qs] * n1_w[None, :]
        kr = qkv_w[1 * D :][qs] * n1_w[None, :]
        vr = qkv_w[2 * D :][qs] * n1_w[None, :]
        ms = slice(r * MID, (r + 1) * MID)
        in_maps.append(
            {
                "x": np.ascontiguousarray(x[b, :T], np.float32),
                "qkw_t": np.ascontiguousarray(
                    np.concatenate([qr, kr], 0).T.astype(BF)
                ),
                "vw_m": np.ascontiguousarray(vr.T.astype(BF)),
                "ow_m": ow_full,
                "w1w_t": np.ascontiguousarray(
                    (w1_w[ms] * n2_w[None, :]).T.astype(BF)
                ),
                "wgw_t": np.ascontiguousarray(
                    (wg_w[ms] * n2_w[None, :]).T.astype(BF)
                ),
                "w2w_m": np.ascontiguousarray(w2_w[:, ms].T.astype(BF)),
                "cosr": cosr,
                "sinr": sinr,
                "cmask": cm,
                "ident": np.eye(128, dtype=BF),
            }
        )
    return in_maps


_CACHE = {}


def _get_nc(T):
    if T not in _CACHE:
        _CACHE[T] = build_nc(T, use_silu=True)
    return _CACHE[T]


def run(inputs, T=2048, trace=False):
    nc = _get_nc(T)
    in_maps = make_in_maps(T=T, **inputs)
    res = run_bass_kernel_spmd(nc, in_maps, core_ids=list(range(8)), trace=trace)
    QT = min(512, T)
    NQ = T // QT
    out = np.empty((B, T, D), dtype=np.float32)
    for b in range(B):
        for r in range(TP):
            shard = res.results[b * TP + r]["out"]  # [NQ*128, D]
            for k in range(NQ):
                # each chunk was reduce-scattered in two halves
                for h in range(2):
                    lo = k * QT + h * 256 + r * 64
                    out[b, lo : lo + 64] = shard[
                        k * 128 + h * 64 : k * 128 + (h + 1) * 64
                    ]
    return out, res


def kernel(**inputs):
    out, _ = run(inputs, T=2048)
    return out
